# revision 2
# baseline (speedup 1.0000x reference)
"""PointGRN (segment_reduce) Trainium2 Bass kernel.

Computation (per segment b, channel c over points feat [N, 64] f32):
    sumsq[b,c]  = sum_{n in seg b} feat[n,c]^2
    r[b,c]      = sqrt(sumsq[b,c])
    rn[b,c]     = r[b,c] / (mean_c r[b,:] + 1e-6)
    out[n,c]    = feat[n,c] * (1 + gamma[c]*rn[b,c]) + beta[c]

Sharding: data-parallel over segments — host reads `offset` and gives each
of the 8 cores one whole segment (padded with zero rows to a 128-row
multiple).  No device-side searchsorted and no collectives needed.

Device kernel (per core), DMA-bound at ~330-350 GB/s/core.  HBM traffic
is the floor: 32 MB in + 32 MB out.  To hit it, pass 1 keeps EVERY tile
resident in SBUF as fp16 (16 MB < 26 MB usable), so pass 2 re-reads
nothing from HBM:
    pass 1: stream [128 x k*64] f32 tiles (k=32 plus one ragged tail) on
            alternating SP/ACT HWDGE rings + SWDGE; DVE converts each
            tile to a resident fp16 copy; ACT squares into bf16; PE
            ones-matmul reduces partitions into 4 PSUM accumulator rows.
    combine: tiny [1,64] vector math (sqrt + Newton step, mean, scale),
            broadcast [s | beta] to [128,128] via a K=1 matmul.
    pass 2: y = fp16_resident * s + beta into f32 staging tiles, stored
            on alternating SP/ACT rings.  Mult runs on DVE; the +beta
            alternates DVE/GPSIMD (11:20 Bresenham) so neither engine
            exceeds the ~92 us store-bound phase.
fp16 rounding of the residual adds ~5e-4 max relative error on the x*s
term — well inside the 2e-2 gate (sumsq still comes from the f32 loads).
"""

import numpy as np

import concourse.bacc as bacc
import concourse.bass as bass
import concourse.mybir as mybir
import concourse.tile as tile
from concourse.bass_utils import run_bass_kernel_spmd

EPS = 1e-06
N_CORES = 8
P = 128          # SBUF partitions
C = 64           # channels
K = 32           # row-groups per partition per full tile
F = K * C        # full-tile free dim (2048 f32 = 8KB/partition)
TILE_ROWS = P * K  # 4096 rows per full tile
MM_N = 512       # matmul moving free-dim chunk
NCHUNK = F // MM_N

_AFT = mybir.ActivationFunctionType
_ALU = mybir.AluOpType

_program_cache: dict[tuple, bass.Bass] = {}


def _tile_rows(r_pad):
    """Split r_pad rows into full [128 x K] tiles plus one ragged tail tile."""
    pchunks = r_pad // P
    nt_full = pchunks // K
    k_tail = pchunks % K
    ks = [K] * nt_full + ([k_tail] if k_tail else [])
    return ks


def _build_program(
    r_pad: int,
    repeats: int = 1,
    dve_adds: int = 11,
    bufs_x: int = 4,
    res_dt=mybir.dt.float16,
) -> bass.Bass:
    """One-core Bass program for a shard of r_pad rows (r_pad % 128 == 0).

    `repeats` re-runs the whole computation body that many times (timing
    only: the wall-clock slope over repeats isolates kernel time from the
    ~100ms flat dispatch overhead of this axon environment).
    `dve_adds` of the nt per-tile +beta ops run on DVE, the rest on GPSIMD.
    """
    from contextlib import ExitStack

    ks = _tile_rows(r_pad)
    nt = len(ks)
    nc = bacc.Bacc()

    feat = nc.declare_dram_parameter("feat", [r_pad, C], mybir.dt.float32, isOutput=False)
    gamma = nc.declare_dram_parameter("gamma", [1, C], mybir.dt.float32, isOutput=False)
    beta = nc.declare_dram_parameter("beta", [1, C], mybir.dt.float32, isOutput=False)
    out = nc.declare_dram_parameter("out", [r_pad, C], mybir.dt.float32, isOutput=True)

    row0 = [0] * nt
    for t in range(1, nt):
        row0[t] = row0[t - 1] + P * ks[t - 1]

    def feat_view(t):
        r0 = row0[t]
        return feat[r0 : r0 + P * ks[t], :].rearrange("(p k) c -> p (k c)", k=ks[t])

    def out_view(t):
        r0 = row0[t]
        return out[r0 : r0 + P * ks[t], :].rearrange("(p k) c -> p (k c)", k=ks[t])

    with tile.TileContext(nc) as tc, ExitStack() as ctx:
        const = ctx.enter_context(tc.tile_pool(name="const", bufs=1))
        inp = ctx.enter_context(tc.tile_pool(name="inp", bufs=bufs_x))
        resp = ctx.enter_context(tc.tile_pool(name="resp", bufs=1))
        sqp = ctx.enter_context(tc.tile_pool(name="sqp", bufs=2))
        psum = ctx.enter_context(tc.tile_pool(name="psum", bufs=1, space="PSUM"))
        small = ctx.enter_context(tc.tile_pool(name="small", bufs=1))

        ones_col = const.tile([P, 1], mybir.dt.bfloat16, name="ones_col", tag="ones_col")
        nc.vector.memset(ones_col, 1.0)
        ones_row = const.tile([1, P], mybir.dt.float32, name="ones_row", tag="ones_row")
        nc.vector.memset(ones_row, 1.0)
        eps_t = const.tile([1, 1], mybir.dt.float32, name="eps_t", tag="eps_t")
        nc.vector.memset(eps_t, EPS)
        g_row = const.tile([1, C], mybir.dt.float32, name="g_row", tag="g_row")
        nc.sync.dma_start(out=g_row, in_=gamma[:])
        b_row = const.tile([1, C], mybir.dt.float32, name="b_row", tag="b_row")
        nc.sync.dma_start(out=b_row, in_=beta[:])

        # chunks actually written, and the last tile writing each (stop flag)
        nchunks = (max(ks) * C + MM_N - 1) // MM_N
        last_t_for_chunk = [0] * nchunks
        for t in range(nt):
            for j in range((ks[t] * C + MM_N - 1) // MM_N):
                last_t_for_chunk[j] = t

        load_eng = [nc.sync, nc.scalar, nc.gpsimd]
        store_eng = [nc.sync, nc.scalar]

        for _rep in range(repeats):
            # --- pass 1: sum of squares; fp16 copy of every tile stays ---
            acc = [
                psum.tile([1, MM_N], mybir.dt.float32, name=f"acc{j}", tag=f"acc{j}")
                for j in range(nchunks)
            ]
            res_tiles = []
            for t in range(nt):
                f_t = ks[t] * C
                x = inp.tile([P, F], mybir.dt.float32, name="x", tag="x")[:, :f_t]
                load_eng[t % 3].dma_start(out=x, in_=feat_view(t))
                h = resp.tile([P, F], res_dt, name="h", tag=f"res{t}")[:, :f_t]
                res_tiles.append(h)
                nc.vector.tensor_copy(h, x)
                sq = sqp.tile([P, F], mybir.dt.bfloat16, name="sq", tag="sq")
                nc.scalar.activation(sq[:, :f_t], x, _AFT.Square)
                for j in range((f_t + MM_N - 1) // MM_N):
                    w = min(MM_N, f_t - j * MM_N)
                    nc.tensor.matmul(
                        acc[j][:, :w],
                        lhsT=ones_col[:, :],
                        rhs=sq[:, j * MM_N : j * MM_N + w],
                        start=(t == 0),
                        stop=(t == last_t_for_chunk[j]),
                    )

            # --- combine: [1,64] vector math ------------------------------
            red = small.tile([1, NCHUNK, C], mybir.dt.float32, name="red", tag="red")
            if nchunks < NCHUNK:
                nc.vector.memset(red[:, :, :], 0.0)
            for j in range(nchunks):
                # a chunk may be only partially covered (ragged tail): reduce
                # the written prefix; zero-init handles the rest
                kw = min(MM_N, max(ks) * C - j * MM_N) // C
                nc.vector.tensor_reduce(
                    out=red[:, j, :],
                    in_=acc[j][:, : kw * C].rearrange("p (k c) -> p c k", c=C),
                    axis=mybir.AxisListType.X,
                    op=_ALU.add,
                )
            sumsq = small.tile([1, C], mybir.dt.float32, name="sumsq", tag="sumsq")
            nc.vector.tensor_reduce(
                out=sumsq,
                in_=red[:, :, :].rearrange("p k c -> p c k"),
                axis=mybir.AxisListType.X,
                op=_ALU.add,
            )

            # r2 = 2*sqrt(sumsq) via ACT sqrt + one Newton step (ACT sqrt is
            # low precision; Newton with the accurate DVE reciprocal fixes it)
            r0 = small.tile([1, C], mybir.dt.float32, name="r0", tag="r0")
            nc.scalar.activation(r0, sumsq, _AFT.Sqrt)
            rm = small.tile([1, C], mybir.dt.float32, name="rm", tag="rm")
            nc.vector.tensor_scalar_max(rm, r0, 1e-30)
            rinv = small.tile([1, C], mybir.dt.float32, name="rinv", tag="rinv")
            nc.vector.reciprocal(rinv, rm)
            t1 = small.tile([1, C], mybir.dt.float32, name="t1", tag="t1")
            nc.vector.tensor_mul(t1, sumsq, rinv)
            r2 = small.tile([1, C], mybir.dt.float32, name="r2", tag="r2")
            nc.vector.tensor_add(r2, r0, t1)

            # mean + eps:  me = sum(r2)/128 + EPS   (r2 = 2r -> mean = sum/128)
            msum = small.tile([1, 1], mybir.dt.float32, name="msum", tag="msum")
            nc.vector.tensor_reduce(out=msum, in_=r2, axis=mybir.AxisListType.X, op=_ALU.add)
            me = small.tile([1, 1], mybir.dt.float32, name="me", tag="me")
            nc.scalar.activation(me, msum, _AFT.Identity, bias=eps_t[:, :], scale=1.0 / (2 * C))
            minv = small.tile([1, 1], mybir.dt.float32, name="minv", tag="minv")
            nc.vector.reciprocal(minv, me)

            # s = 1 + gamma * (r2 * 0.5 * minv); pack [s | beta] in one row
            t2 = small.tile([1, C], mybir.dt.float32, name="t2", tag="t2")
            nc.vector.tensor_mul(t2, r2, g_row)
            mh = small.tile([1, 1], mybir.dt.float32, name="mh", tag="mh")
            nc.vector.tensor_scalar_mul(mh, minv, 0.5)
            sb_cat = small.tile([1, 2 * C], mybir.dt.float32, name="sb_cat", tag="sb_cat")
            # beta half has no pass-1 deps: scheduler can issue it early
            nc.vector.tensor_copy(sb_cat[:, C : 2 * C], b_row)
            nc.vector.tensor_scalar(
                sb_cat[:, 0:C], t2, scalar1=mh[:, :], scalar2=1.0, op0=_ALU.mult, op1=_ALU.add
            )

            # broadcast [1,128] -> [128,128]: cols 0-63 = s, 64-127 = beta
            bc_ps = psum.tile([P, 2 * C], mybir.dt.float32, name="bc_ps", tag="bc_ps")
            nc.tensor.matmul(bc_ps[:, :], lhsT=ones_row[:, :], rhs=sb_cat[:, :], start=True, stop=True)
            sb_bc = small.tile([P, 2 * C], mybir.dt.float32, name="sb_bc", tag="sb_bc")
            nc.scalar.copy(sb_bc, bc_ps)
            s_bc = sb_bc[:, 0:C]
            b_bc = sb_bc[:, C : 2 * C]

            def bcast_ap(col_slice, kk):
                return bass.AP(
                    tensor=col_slice.tensor,
                    offset=col_slice.offset,
                    ap=[col_slice.ap[0], [0, kk], col_slice.ap[1]],
                )

            # --- pass 2: y = h*s + beta into f32 staging, store -----------
            dve_add_mark = 0
            for t in range(nt):
                kk = ks[t]
                f_t = kk * C
                h3 = res_tiles[t].rearrange("p (k c) -> p k c", c=C)
                y = inp.tile([P, F], mybir.dt.float32, name="x", tag="x")[:, :f_t]
                y3 = y.rearrange("p (k c) -> p k c", c=C)
                nc.vector.tensor_tensor(y3, h3, bcast_ap(s_bc, kk), _ALU.mult)
                # Bresenham dve_adds/nt interleave of +beta onto DVE
                nxt = ((t + 1) * dve_adds) // nt
                eng = nc.vector if nxt != dve_add_mark else nc.gpsimd
                dve_add_mark = nxt
                eng.tensor_tensor(y3, y3, bcast_ap(b_bc, kk), _ALU.add)
                store_eng[t % 2].dma_start(out=out_view(t), in_=y)

    nc.finalize()
    return nc


def kernel(feat: np.ndarray, offset: np.ndarray, gamma: np.ndarray, beta: np.ndarray) -> np.ndarray:
    feat = np.ascontiguousarray(np.asarray(feat, dtype=np.float32))
    offset = np.asarray(offset)
    gamma = np.ascontiguousarray(np.asarray(gamma, dtype=np.float32)).reshape(1, C)
    beta = np.ascontiguousarray(np.asarray(beta, dtype=np.float32)).reshape(1, C)

    n = feat.shape[0]
    b = offset.shape[0]
    assert b <= N_CORES, f"need <= {N_CORES} segments, got {b}"

    ends = offset.astype(np.int64)
    starts = np.concatenate([[0], ends[:-1]])
    seg_rows = (ends - starts).astype(np.int64)

    r_max = int(seg_rows.max()) if b else P
    r_pad = max(P, ((r_max + P - 1) // P) * P)

    key = (r_pad,)
    nc = _program_cache.get(key)
    if nc is None:
        nc = _build_program(r_pad)
        _program_cache[key] = nc

    in_maps = []
    for i in range(N_CORES):
        shard = np.zeros((r_pad, C), dtype=np.float32)
        if i < b and seg_rows[i] > 0:
            shard[: seg_rows[i]] = feat[starts[i] : ends[i]]
        in_maps.append({"feat": shard, "gamma": gamma, "beta": beta})

    results = run_bass_kernel_spmd(nc, in_maps, core_ids=list(range(N_CORES))).results

    out_full = np.empty((n, C), dtype=np.float32)
    for i in range(b):
        if seg_rows[i] > 0:
            out_full[starts[i] : ends[i]] = results[i]["out"][: seg_rows[i]]

    # Rows past offset[-1] (possible with general sorted offsets): the
    # reference's searchsorted yields index b there, which jax clamps to
    # b-1 on gather — those rows are scaled by the last segment's rn but
    # excluded from its sumsq.  Replicate on host.
    tail0 = int(ends[-1]) if b else 0
    if tail0 < n:
        last0, last1 = int(starts[-1]), int(ends[-1])
        sumsq = (feat[last0:last1].astype(np.float64) ** 2).sum(axis=0)
        r = np.sqrt(sumsq)
        rn = (r / (r.mean() + EPS)).astype(np.float32)
        ft = feat[tail0:]
        out_full[tail0:] = ft + gamma * (ft * rn[None, :]) + beta
    return out_full


# revision 38
# speedup vs baseline: 9.9761x; 9.9761x over previous
"""PointGRN (segment_reduce) Trainium2 Bass kernel.

Computation (per segment b, channel c over points feat [N, 64] f32):
    sumsq[b,c]  = sum_{n in seg b} feat[n,c]^2
    r[b,c]      = sqrt(sumsq[b,c])
    rn[b,c]     = r[b,c] / (mean_c r[b,:] + 1e-6)
    out[n,c]    = feat[n,c] * (1 + gamma[c]*rn[b,c]) + beta[c]

Sharding: data-parallel over segments — host reads `offset` and gives each
of the 8 cores one whole segment (padded with zero rows to a 128-row
multiple).  No device-side searchsorted and no collectives needed.

Device kernel (per core), HBM-traffic floor: 32 MB in + 32 MB out.  Pass 1
keeps EVERY tile resident in SBUF as fp16 (16 MB < 26 MB usable), so pass
2 re-reads nothing from HBM:
    pass 1: stream [128 x k*64] f32 tiles (k=32 plus one ragged tail), all
            on the SWDGE (gpsimd) queue — measured faster than any HWDGE
            mix here; DVE converts each tile to a resident fp16 copy
            (2x packed mode); ACT squares into bf16; PE ones-matmul
            reduces partitions into one 4-bank PSUM accumulator row.
    combine: short serial chain: one DVE reduce of the PSUM row -> ACT
            sqrt with accum_out (mean for free) -> max-guard -> DVE
            reciprocal -> fused scale+1 -> [s | beta] broadcast to
            [128,128] via a K=1 matmul (all in fp16 for pass 2).
    pass 2: w = h * s + beta entirely in fp16 on DVE (both tensor_tensor
            ops hit the 2x packed mode, ~66 us total vs 130 us in f32).
            21/31 tiles store via casting SWDGE DMAs (fp16 SBUF -> f32
            HBM, no upcast op at all); the rest upcast on the idle ACT
            engine and store on the SP/ACT HWDGE rings, which balances
            the three DMA queues.
Engine-assignment findings baked in: GPSIMD tensor_tensor is ~2x slower
than DVE and mixing it into the add stream serializes badly (+50 us);
scalar(ACT)-ring DMAs stall whenever ACT has compute queued, so loads
never use it and stores only when ACT is nearly idle.
fp16 rounding (resident + staging) gives ~3e-4 median / ~1.2e-3 absmax
relative error — well inside the 2e-2 gate (sumsq still comes from the
f32 loads; the combine chain is exact f32).
"""

import numpy as np

import concourse.bacc as bacc
import concourse.bass as bass
import concourse.mybir as mybir
import concourse.tile as tile
from concourse.bass_utils import run_bass_kernel_spmd

EPS = 1e-06
N_CORES = 8
P = 128          # SBUF partitions
C = 64           # channels
K = 32           # row-groups per partition per full tile
F = K * C        # full-tile free dim (2048 f32 = 8KB/partition)
TILE_ROWS = P * K  # 4096 rows per full tile
MM_N = 512       # matmul moving free-dim chunk
NCHUNK = F // MM_N

_AFT = mybir.ActivationFunctionType
_ALU = mybir.AluOpType

_program_cache: dict[tuple, bass.Bass] = {}


def _tile_rows(r_pad, k_rows=K):
    """Split r_pad rows into full [128 x k] tiles plus one ragged tail tile."""
    pchunks = r_pad // P
    nt_full = pchunks // k_rows
    k_tail = pchunks % k_rows
    ks = [k_rows] * nt_full + ([k_tail] if k_tail else [])
    return ks


def _build_program(
    r_pad: int,
    repeats: int = 1,
    dve_adds: int = 31,
    bufs_x: int = 4,
    res_dt=mybir.dt.float16,
    load_mix=("gpsimd",),
    store_mix=("sync", "scalar"),
    split_dma: bool = False,
    p1_only: bool = False,
    combine_trim: bool = True,
    conv_eng: str = "vector",
    fp16_stage: bool = True,
    upcast_eng: str = "scalar",
    k_rows: int = K,
    store_split: bool = False,
    cast_stores: int = 21,
    bufs_w: int = 3,
) -> bass.Bass:
    """One-core Bass program for a shard of r_pad rows (r_pad % 128 == 0).

    `repeats` re-runs the whole computation body that many times (timing
    only: the wall-clock slope over repeats isolates kernel time from the
    ~100ms flat dispatch overhead of this axon environment).
    `dve_adds` of the nt per-tile +beta ops run on DVE, the rest on GPSIMD.
    """
    from contextlib import ExitStack

    ks = _tile_rows(r_pad, k_rows)
    nt = len(ks)
    FT = k_rows * C
    nc = bacc.Bacc()

    feat = nc.declare_dram_parameter("feat", [r_pad, C], mybir.dt.float32, isOutput=False)
    gamma = nc.declare_dram_parameter("gamma", [1, C], mybir.dt.float32, isOutput=False)
    beta = nc.declare_dram_parameter("beta", [1, C], mybir.dt.float32, isOutput=False)
    out = nc.declare_dram_parameter("out", [r_pad, C], mybir.dt.float32, isOutput=True)

    row0 = [0] * nt
    for t in range(1, nt):
        row0[t] = row0[t - 1] + P * ks[t - 1]

    def feat_view(t):
        r0 = row0[t]
        return feat[r0 : r0 + P * ks[t], :].rearrange("(p k) c -> p (k c)", k=ks[t])

    def out_view(t):
        r0 = row0[t]
        return out[r0 : r0 + P * ks[t], :].rearrange("(p k) c -> p (k c)", k=ks[t])

    with tile.TileContext(nc) as tc, ExitStack() as ctx:
        const = ctx.enter_context(tc.tile_pool(name="const", bufs=1))
        inp = ctx.enter_context(tc.tile_pool(name="inp", bufs=bufs_x))
        resp = ctx.enter_context(tc.tile_pool(name="resp", bufs=1))
        sqp = ctx.enter_context(tc.tile_pool(name="sqp", bufs=2))
        wp = ctx.enter_context(tc.tile_pool(name="wp", bufs=bufs_w)) if fp16_stage else None
        psum = ctx.enter_context(tc.tile_pool(name="psum", bufs=1, space="PSUM"))
        small = ctx.enter_context(tc.tile_pool(name="small", bufs=1))

        ones_col = const.tile([P, 1], mybir.dt.bfloat16, name="ones_col", tag="ones_col")
        nc.vector.memset(ones_col, 1.0)
        ones_row = const.tile([1, P], mybir.dt.float32, name="ones_row", tag="ones_row")
        nc.vector.memset(ones_row, 1.0)
        eps_t = const.tile([1, 1], mybir.dt.float32, name="eps_t", tag="eps_t")
        nc.vector.memset(eps_t, EPS)
        g_row = const.tile([1, C], mybir.dt.float32, name="g_row", tag="g_row")
        nc.sync.dma_start(out=g_row, in_=gamma[:])
        b_row = const.tile([1, C], mybir.dt.float32, name="b_row", tag="b_row")
        nc.sync.dma_start(out=b_row, in_=beta[:])
        # gamma * C, so s = 1 + (gamma*C) * r * (1/sum(r)) needs no /C for the mean
        gC_row = const.tile([1, C], mybir.dt.float32, name="gC_row", tag="gC_row")
        nc.vector.tensor_scalar_mul(gC_row, g_row, float(C))

        # chunks actually written, and the last tile writing each (stop flag)
        nchunks = (max(ks) * C + MM_N - 1) // MM_N
        last_t_for_chunk = [0] * nchunks
        for t in range(nt):
            for j in range((ks[t] * C + MM_N - 1) // MM_N):
                last_t_for_chunk[j] = t

        load_eng = [getattr(nc, e) for e in load_mix]
        store_eng = [getattr(nc, e) for e in store_mix]

        for _rep in range(repeats):
            # --- pass 1: sum of squares; fp16 copy of every tile stays ---
            if combine_trim:
                acc_all = psum.tile(
                    [1, nchunks * MM_N], mybir.dt.float32, name="acc", tag="acc"
                )
                acc = [acc_all[:, j * MM_N : (j + 1) * MM_N] for j in range(nchunks)]
            else:
                acc = [
                    psum.tile([1, MM_N], mybir.dt.float32, name=f"acc{j}", tag=f"acc{j}")
                    for j in range(nchunks)
                ]
            res_tiles = []
            for t in range(nt):
                f_t = ks[t] * C
                x = inp.tile([P, FT], mybir.dt.float32, name="x", tag="x")[:, :f_t]
                if split_dma:
                    half = (f_t // 2 + C - 1) // C * C
                    fv = feat_view(t)
                    load_eng[0].dma_start(out=x[:, :half], in_=fv[:, :half])
                    load_eng[1].dma_start(out=x[:, half:], in_=fv[:, half:])
                else:
                    load_eng[t % len(load_eng)].dma_start(out=x, in_=feat_view(t))
                h = resp.tile([P, FT], res_dt, name="h", tag=f"res{t}")[:, :f_t]
                res_tiles.append(h)
                if conv_eng == "scalar":
                    nc.scalar.copy(h, x)
                else:
                    getattr(nc, conv_eng).tensor_copy(h, x)
                sq = sqp.tile([P, FT], mybir.dt.bfloat16, name="sq", tag="sq")
                nc.scalar.activation(sq[:, :f_t], x, _AFT.Square)
                for j in range((f_t + MM_N - 1) // MM_N):
                    w = min(MM_N, f_t - j * MM_N)
                    nc.tensor.matmul(
                        acc[j][:, :w],
                        lhsT=ones_col[:, :],
                        rhs=sq[:, j * MM_N : j * MM_N + w],
                        start=(t == 0),
                        stop=(t == last_t_for_chunk[j]),
                    )

            # --- combine: [1,64] vector math ------------------------------
            if combine_trim:
                # critical path: reduce -> sqrt(+accum) -> max -> recip -> ts
                #                -> bcast matmul -> psum copy
                sumsq = small.tile([1, C], mybir.dt.float32, name="sumsq", tag="sumsq")
                nc.vector.tensor_reduce(
                    out=sumsq,
                    in_=acc_all[:, :].rearrange("p (k c) -> p c k", c=C),
                    axis=mybir.AxisListType.X,
                    op=_ALU.add,
                )
                r_t = small.tile([1, C], mybir.dt.float32, name="r_t", tag="r_t")
                msum = small.tile([1, 1], mybir.dt.float32, name="msum", tag="msum")
                nc.scalar.activation(r_t, sumsq, _AFT.Sqrt, accum_out=msum)
                msafe = small.tile([1, 1], mybir.dt.float32, name="msafe", tag="msafe")
                nc.vector.tensor_scalar_max(msafe, msum, 1e-28)
                minv = small.tile([1, 1], mybir.dt.float32, name="minv", tag="minv")
                nc.vector.reciprocal(minv, msafe)
                t2 = small.tile([1, C], mybir.dt.float32, name="t2", tag="t2")
                nc.vector.tensor_mul(t2, r_t, gC_row)
                sb_cat = small.tile([1, 2 * C], mybir.dt.float32, name="sb_cat", tag="sb_cat")
                nc.vector.tensor_copy(sb_cat[:, C : 2 * C], b_row)
                nc.vector.tensor_scalar(
                    sb_cat[:, 0:C], t2, scalar1=minv[:, :], scalar2=1.0,
                    op0=_ALU.mult, op1=_ALU.add,
                )
                bc_ps = psum.tile([P, 2 * C], mybir.dt.float32, name="bc_ps", tag="bc_ps")
                nc.tensor.matmul(bc_ps[:, :], lhsT=ones_row[:, :], rhs=sb_cat[:, :], start=True, stop=True)
                sb_dt = mybir.dt.float16 if fp16_stage else mybir.dt.float32
                sb_bc = small.tile([P, 2 * C], sb_dt, name="sb_bc", tag="sb_bc")
                nc.scalar.copy(sb_bc, bc_ps)
                s_bc = sb_bc[:, 0:C]
                b_bc = sb_bc[:, C : 2 * C]
            else:
                red = small.tile([1, max(nchunks, 1), C], mybir.dt.float32, name="red", tag="red")
                if False:
                    nc.vector.memset(red[:, :, :], 0.0)
                for j in range(nchunks):
                    # a chunk may be only partially covered (ragged tail):
                    # reduce the written prefix; zero-init handles the rest
                    kw = min(MM_N, max(ks) * C - j * MM_N) // C
                    nc.vector.tensor_reduce(
                        out=red[:, j, :],
                        in_=acc[j][:, : kw * C].rearrange("p (k c) -> p c k", c=C),
                        axis=mybir.AxisListType.X,
                        op=_ALU.add,
                    )
                sumsq = small.tile([1, C], mybir.dt.float32, name="sumsq", tag="sumsq")
                nc.vector.tensor_reduce(
                    out=sumsq,
                    in_=red[:, :, :].rearrange("p k c -> p c k"),
                    axis=mybir.AxisListType.X,
                    op=_ALU.add,
                )

                # r2 = 2*sqrt(sumsq) via ACT sqrt + one Newton step
                r0 = small.tile([1, C], mybir.dt.float32, name="r0", tag="r0")
                nc.scalar.activation(r0, sumsq, _AFT.Sqrt)
                rm = small.tile([1, C], mybir.dt.float32, name="rm", tag="rm")
                nc.vector.tensor_scalar_max(rm, r0, 1e-30)
                rinv = small.tile([1, C], mybir.dt.float32, name="rinv", tag="rinv")
                nc.vector.reciprocal(rinv, rm)
                t1 = small.tile([1, C], mybir.dt.float32, name="t1", tag="t1")
                nc.vector.tensor_mul(t1, sumsq, rinv)
                r2 = small.tile([1, C], mybir.dt.float32, name="r2", tag="r2")
                nc.vector.tensor_add(r2, r0, t1)

                # mean + eps:  me = sum(r2)/128 + EPS  (r2 = 2r)
                msum = small.tile([1, 1], mybir.dt.float32, name="msum", tag="msum")
                nc.vector.tensor_reduce(out=msum, in_=r2, axis=mybir.AxisListType.X, op=_ALU.add)
                me = small.tile([1, 1], mybir.dt.float32, name="me", tag="me")
                nc.scalar.activation(me, msum, _AFT.Identity, bias=eps_t[:, :], scale=1.0 / (2 * C))
                minv = small.tile([1, 1], mybir.dt.float32, name="minv", tag="minv")
                nc.vector.reciprocal(minv, me)

                # s = 1 + gamma * (r2 * 0.5 * minv); pack [s | beta] in one row
                t2 = small.tile([1, C], mybir.dt.float32, name="t2", tag="t2")
                nc.vector.tensor_mul(t2, r2, g_row)
                mh = small.tile([1, 1], mybir.dt.float32, name="mh", tag="mh")
                nc.vector.tensor_scalar_mul(mh, minv, 0.5)
                sb_cat = small.tile([1, 2 * C], mybir.dt.float32, name="sb_cat", tag="sb_cat")
                # beta half has no pass-1 deps: scheduler can issue it early
                nc.vector.tensor_copy(sb_cat[:, C : 2 * C], b_row)
                nc.vector.tensor_scalar(
                    sb_cat[:, 0:C], t2, scalar1=mh[:, :], scalar2=1.0, op0=_ALU.mult, op1=_ALU.add
                )

                # broadcast [1,128] -> [128,128]: cols 0-63 = s, 64-127 = beta
                bc_ps = psum.tile([P, 2 * C], mybir.dt.float32, name="bc_ps", tag="bc_ps")
                nc.tensor.matmul(bc_ps[:, :], lhsT=ones_row[:, :], rhs=sb_cat[:, :], start=True, stop=True)
                sb_bc = small.tile([P, 2 * C], mybir.dt.float32, name="sb_bc", tag="sb_bc")
                nc.scalar.copy(sb_bc, bc_ps)
                s_bc = sb_bc[:, 0:C]
                b_bc = sb_bc[:, C : 2 * C]

            def bcast_ap(col_slice, kk):
                return bass.AP(
                    tensor=col_slice.tensor,
                    offset=col_slice.offset,
                    ap=[col_slice.ap[0], [0, kk], col_slice.ap[1]],
                )

            # --- pass 2: y = h*s + beta into f32 staging, store -----------
            if p1_only:
                # decomposition probe: store sb_bc so combine isn't dead-code
                nc.sync.dma_start(out=out[0:P, :], in_=sb_bc[:, 0:C])
                continue
            dve_add_mark = 0
            cast_mark = 0
            ns = 0
            for t in range(nt):
                kk = ks[t]
                f_t = kk * C
                h3 = res_tiles[t].rearrange("p (k c) -> p k c", c=C)
                if fp16_stage and cast_stores:
                    # Bresenham cast_stores/nt of tiles skip the upcast and
                    # store via a casting SWDGE DMA (fp16 SBUF -> f32 HBM)
                    nxt = ((t + 1) * cast_stores) // nt
                    use_cast = nxt != cast_mark
                    cast_mark = nxt
                    w = wp.tile([P, FT], mybir.dt.float16, name="w", tag="w")[:, :f_t]
                    w3 = w.rearrange("p (k c) -> p k c", c=C)
                    nc.vector.tensor_tensor(w3, h3, bcast_ap(s_bc, kk), _ALU.mult)
                    nc.vector.tensor_tensor(w3, w3, bcast_ap(b_bc, kk), _ALU.add)
                    if use_cast:
                        nc.gpsimd.dma_start(out=out_view(t), in_=w)
                    else:
                        y = inp.tile([P, FT], mybir.dt.float32, name="x", tag="x")[:, :f_t]
                        nc.scalar.copy(y, w)
                        store_eng[ns % len(store_eng)].dma_start(out=out_view(t), in_=y)
                        ns += 1
                    continue
                y = inp.tile([P, FT], mybir.dt.float32, name="x", tag="x")[:, :f_t]
                if fp16_stage:
                    # mult+add fully 16-bit (DVE 2x packed mode), then the
                    # idle upcast engine widens to the f32 store tile
                    w = wp.tile([P, FT], mybir.dt.float16, name="w", tag="w")[:, :f_t]
                    w3 = w.rearrange("p (k c) -> p k c", c=C)
                    nc.vector.tensor_tensor(w3, h3, bcast_ap(s_bc, kk), _ALU.mult)
                    nc.vector.tensor_tensor(w3, w3, bcast_ap(b_bc, kk), _ALU.add)
                    if upcast_eng == "mixed":
                        # DVE takes every 3rd upcast so the ACT ring can
                        # carry stores for those tiles
                        if t % 3 == 2:
                            nc.vector.tensor_copy(y, w)
                        else:
                            nc.scalar.copy(y, w)
                    elif upcast_eng == "scalar":
                        nc.scalar.copy(y, w)
                    else:
                        getattr(nc, upcast_eng).tensor_copy(y, w)
                else:
                    y3 = y.rearrange("p (k c) -> p k c", c=C)
                    nc.vector.tensor_tensor(y3, h3, bcast_ap(s_bc, kk), _ALU.mult)
                    # Bresenham dve_adds/nt interleave of +beta onto DVE
                    nxt = ((t + 1) * dve_adds) // nt
                    eng = nc.vector if nxt != dve_add_mark else nc.gpsimd
                    dve_add_mark = nxt
                    eng.tensor_tensor(y3, y3, bcast_ap(b_bc, kk), _ALU.add)
                if split_dma or store_split:
                    half = (f_t // 2 + C - 1) // C * C
                    ov = out_view(t)
                    store_eng[0].dma_start(out=ov[:, :half], in_=y[:, :half])
                    store_eng[1].dma_start(out=ov[:, half:], in_=y[:, half:])
                else:
                    store_eng[t % len(store_eng)].dma_start(out=out_view(t), in_=y)

    nc.finalize()
    return nc


def kernel(feat: np.ndarray, offset: np.ndarray, gamma: np.ndarray, beta: np.ndarray) -> np.ndarray:
    feat = np.ascontiguousarray(np.asarray(feat, dtype=np.float32))
    offset = np.asarray(offset)
    gamma = np.ascontiguousarray(np.asarray(gamma, dtype=np.float32)).reshape(1, C)
    beta = np.ascontiguousarray(np.asarray(beta, dtype=np.float32)).reshape(1, C)

    n = feat.shape[0]
    b = offset.shape[0]
    assert b <= N_CORES, f"need <= {N_CORES} segments, got {b}"

    ends = offset.astype(np.int64)
    starts = np.concatenate([[0], ends[:-1]])
    seg_rows = (ends - starts).astype(np.int64)

    r_max = int(seg_rows.max()) if b else P
    r_pad = max(P, ((r_max + P - 1) // P) * P)

    key = (r_pad,)
    nc = _program_cache.get(key)
    if nc is None:
        nc = _build_program(r_pad)
        _program_cache[key] = nc

    in_maps = []
    for i in range(N_CORES):
        shard = np.zeros((r_pad, C), dtype=np.float32)
        if i < b and seg_rows[i] > 0:
            shard[: seg_rows[i]] = feat[starts[i] : ends[i]]
        in_maps.append({"feat": shard, "gamma": gamma, "beta": beta})

    results = run_bass_kernel_spmd(nc, in_maps, core_ids=list(range(N_CORES))).results

    out_full = np.empty((n, C), dtype=np.float32)
    for i in range(b):
        if seg_rows[i] > 0:
            out_full[starts[i] : ends[i]] = results[i]["out"][: seg_rows[i]]

    # Rows past offset[-1] (possible with general sorted offsets): the
    # reference's searchsorted yields index b there, which jax clamps to
    # b-1 on gather — those rows are scaled by the last segment's rn but
    # excluded from its sumsq.  Replicate on host.
    tail0 = int(ends[-1]) if b else 0
    if tail0 < n:
        last0, last1 = int(starts[-1]), int(ends[-1])
        sumsq = (feat[last0:last1].astype(np.float64) ** 2).sum(axis=0)
        r = np.sqrt(sumsq)
        rn = (r / (r.mean() + EPS)).astype(np.float32)
        ft = feat[tail0:]
        out_full[tail0:] = ft + gamma * (ft * rn[None, :]) + beta
    return out_full
